# revision 24
# baseline (speedup 1.0000x reference)
"""Trainium2 Bass kernel for fused LoRA linear with per-sequence adapter routing.

Problem (hardcoded shapes):
  x [8192, 4096] fp32, base_weight [4096, 4096], a_cache/b_cache [512, 4096],
  16 sequences x 512 tokens, 8 adapters (rank <= 64), out [8192, 4096]:
      out = x @ base_weight.T + scaling[a(t)] * (x @ A[a(t)].T masked) @ B[a(t)]

Sharding: data-parallel over tokens. Core c handles sequences {2c, 2c+1}
(tokens [1024c, 1024c+1024)) and computes the full 4096 output features for
its tokens. Host-side prep gathers/masks/scales the per-sequence LoRA weights,
converts x/W/B to bf16, computes the tiny rank-reduction xa = x @ A.T
(<2% of total FLOPs; the 274-GFLOP base GEMM + rank-expansion stay on
device), and lays every DRAM tensor out in its exact SBUF layout so each DMA
is one large fully-contiguous transfer.

Device schedule (per core):
  - SP queue: xa (1 DMA), bs (1 DMA), xT (8 DMAs of 4 k-tiles).
  - Activation queue: w chunk stream (2-slot SBUF ring; chunk 0 split in 4
    pieces gated on xT arrival so startup loads aren't starved on the shared
    SDMA engines) interleaved with the per-chunk output stores.
  - PE: every output chunk accumulates lora first (xaT.T @ bs) then the
    32 k-tiles of the base GEMM. Chunk 0 runs k-outer across all 8 PSUM
    banks, consuming xT groups as they arrive; chunks 1..7 run j-outer/
    k-inner so each PSUM bank's stop lands ~7us before the next chunk
    reopens it -> the DVE drain copies never stall the PE.
  - DVE: PSUM -> SBUF output copies (fp32).

All matmuls bf16 (1 cycle/row, fast weight load), fp32 PSUM accumulation.
Semaphore discipline: every individually-awaited DMA has its own semaphore
(partial counts on a shared semaphore race across the 16 per-engine
increments); shared-semaphore waits only use full counts.
"""
import numpy as np

import concourse.bass as bass
import concourse.mybir as mybir
from concourse.bass_utils import run_bass_kernel_spmd

P = 128
NCORES = 8
T_CORE = 1024            # tokens per core (2 sequences)
K = 4096                 # in features
N = 4096                 # out features
KT = K // P              # 32 k-tiles
NCHUNK = 512             # psum free dim per matmul
NC_N = N // NCHUNK       # 8 n-chunks
TT = T_CORE // P         # 8 t-tiles per core
SEQ_LEN = 512
MAX_RANK = 64
KG = 4                   # k-tiles per xT DMA group
NG = KT // KG            # 8 xT groups
WSLOT = KT * NCHUNK      # w ring slot width (one n-chunk, all k)
WPIECE = WSLOT // 4      # chunk-0 w DMA piece (8 k-tiles)

F32 = mybir.dt.float32
BF16 = mybir.dt.bfloat16
NP_BF16 = mybir.dt.np(BF16)

_PROGRAM = None  # cached (nc,) build


def _build_program():
    nc = bass.Bass()
    xt_d = nc.dram_tensor("xt", [P, KT * T_CORE], BF16, kind="ExternalInput")
    wt_d = nc.dram_tensor("wt", [P, NC_N * WSLOT], BF16, kind="ExternalInput")
    xad_d = nc.dram_tensor("xad", [P, T_CORE], BF16, kind="ExternalInput")
    bs_d = nc.dram_tensor("bs", [P, N], BF16, kind="ExternalInput")
    out_d = nc.dram_tensor("out", [P, NC_N * TT * NCHUNK], F32, kind="ExternalOutput")

    from contextlib import ExitStack
    with ExitStack() as ctx:
        e = ctx.enter_context
        xT_s = e(nc.sbuf_tensor("xT_s", [P, KT * T_CORE], BF16))   # 64 KB/part
        w_s = e(nc.sbuf_tensor("w_s", [P, 2 * WSLOT], BF16))       # 64 KB/part
        bs_s = e(nc.sbuf_tensor("bs_s", [P, N], BF16))             # 8 KB/part
        xaT_s = e(nc.sbuf_tensor("xaT_s", [P, T_CORE], BF16))      # 2 KB/part
        os_s = e(nc.sbuf_tensor("os_s", [P, TT * NCHUNK], F32))    # 16 KB/part
        banks = [e(nc.psum_tensor(f"pbank{i}", [P, NCHUNK], F32)) for i in range(8)]
        s_xad = e(nc.semaphore("s_xad"))
        xt_sems = [e(nc.semaphore(f"s_xt{i}")) for i in range(NG)]
        s_bs = e(nc.semaphore("s_bs"))
        w_sems = [e(nc.semaphore(f"s_w{i}")) for i in range(2)]
        wp_sems = [e(nc.semaphore(f"s_wp{i}")) for i in range(4)]
        s_bank = e(nc.semaphore("s_bank"))
        s_cp = e(nc.semaphore("s_cp"))
        s_od = e(nc.semaphore("s_od"))
        block = e(nc.Block(no_gpsimd_drain=True))  # gpsimd entirely unused

        def xtile(k, j):
            return xT_s[:, k * T_CORE + j * P:k * T_CORE + (j + 1) * P]

        def wsl(c, k):
            base = (c % 2) * WSLOT + k * NCHUNK
            return w_s[:, base:base + NCHUNK]

        def wslot_ready(c):
            # w_sems value guaranteeing chunk c resident: slot 0 gets chunks
            # 2,4,6 (chunk 0 arrives via wp_sems pieces); slot 1 gets 1,3,5,7.
            if c % 2 == 0:
                return 16 * (c // 2)
            return 16 * ((c + 1) // 2)

        @block.sync
        def _(sync):
            sync.dma_start(out=xaT_s[:], in_=xad_d[:]).then_inc(s_xad, 16)
            sync.dma_start(out=bs_s[:], in_=bs_d[:]).then_inc(s_bs, 16)
            gw = KG * T_CORE
            for g in range(NG):
                sync.dma_start(
                    out=xT_s[:, g * gw:(g + 1) * gw],
                    in_=xt_d[:, g * gw:(g + 1) * gw],
                ).then_inc(xt_sems[g], 16)

        @block.scalar
        def _(scalar):
            # w chunk stream (2-slot ring) interleaved with output stores.
            # chunk 0 in 4 pieces, gated so the startup-critical loads aren't
            # starved on the shared SDMA engines.
            for i in range(4):
                if i > 0:
                    scalar.wait_ge(xt_sems[2 * i - 1], 16)
                scalar.dma_start(
                    out=w_s[:, i * WPIECE:(i + 1) * WPIECE],
                    in_=wt_d[:, i * WPIECE:(i + 1) * WPIECE],
                ).then_inc(wp_sems[i], 16)
            scalar.wait_ge(xt_sems[NG - 1], 16)
            ow = TT * NCHUNK
            hw_ = ow // 2

            def store(cc, piece):
                scalar.wait_ge(s_cp, cc * TT + 4 * (piece + 1))
                scalar.dma_start(
                    out=out_d[:, cc * ow + piece * hw_:cc * ow + (piece + 1) * hw_],
                    in_=os_s[:, piece * hw_:(piece + 1) * hw_],
                ).then_inc(s_od, 16)

            for c in range(1, NC_N):
                if c >= 2:
                    # slot's previous occupant (chunk c-2) fully drained
                    scalar.wait_ge(s_cp, (c - 1) * TT)
                scalar.dma_start(
                    out=w_s[:, (c % 2) * WSLOT:(c % 2 + 1) * WSLOT],
                    in_=wt_d[:, c * WSLOT:(c + 1) * WSLOT],
                ).then_inc(w_sems[c % 2], 16)
                if c >= 2:
                    store(c - 2, 0)
                    store(c - 2, 1)
            store(NC_N - 2, 0)
            store(NC_N - 2, 1)
            # last chunk: 8 finer pieces to shorten the tail
            qw = ow // 8
            cc = NC_N - 1
            for pq in range(8):
                scalar.wait_ge(s_cp, cc * TT + pq + 1)
                scalar.dma_start(
                    out=out_d[:, cc * ow + pq * qw:cc * ow + (pq + 1) * qw],
                    in_=os_s[:, pq * qw:(pq + 1) * qw],
                ).then_inc(s_od, 16)

        @block.tensor
        def _(tensor):
            # ---- chunk 0: lora first, then k-outer across all 8 banks,
            # consuming xT groups as they arrive ----
            tensor.wait_ge(s_xad, 16)
            # HAM warmup: a few throwaway matmuls on already-resident data
            # while waiting for bs, so the PE clock gate is released before
            # the real stream starts. Bank 0 is reopened with start=True by
            # the first lora matmul, so the results are discarded.
            for _ in range(4):
                tensor.matmul(banks[0][:], lhsT=xaT_s[:, 0:P],
                              rhs=xaT_s[:, 0:NCHUNK], start=True, stop=True)
            tensor.wait_ge(s_bs, 16)
            for j in range(TT):
                tensor.matmul(
                    banks[j][:], lhsT=xaT_s[:, j * P:(j + 1) * P],
                    rhs=bs_s[:, 0:NCHUNK], start=True, stop=False)
            for g in range(NG):
                tensor.wait_ge(xt_sems[g], 16)
                if g % 2 == 0:
                    tensor.wait_ge(wp_sems[g // 2], 16)
                for k in range(g * KG, (g + 1) * KG):
                    for j in range(TT):
                        mm = tensor.matmul(
                            banks[j][:], lhsT=xtile(k, j), rhs=wsl(0, k),
                            start=False, stop=(k == KT - 1))
                        if k == KT - 1:
                            mm.then_inc(s_bank, 1)

            # ---- chunks 1..7 steady state: j-outer / k-inner ----
            for c in range(1, NC_N):
                tensor.wait_ge(w_sems[c % 2], wslot_ready(c))
                for j in range(TT):
                    tensor.wait_ge(s_cp, (c - 1) * TT + j + 1)
                    tensor.matmul(
                        banks[j][:], lhsT=xaT_s[:, j * P:(j + 1) * P],
                        rhs=bs_s[:, c * NCHUNK:(c + 1) * NCHUNK],
                        start=True, stop=False)
                    for k in range(KT):
                        mm = tensor.matmul(
                            banks[j][:], lhsT=xtile(k, j), rhs=wsl(c, k),
                            start=False, stop=(k == KT - 1))
                    mm.then_inc(s_bank, 1)

        @block.vector
        def _(vector):
            # out copies psum -> staging
            for c in range(NC_N):
                for j in range(TT):
                    vector.wait_ge(s_bank, c * TT + j + 1)
                    if c >= 1 and j == 0:
                        # all stores through chunk c-1 done (full-count wait:
                        # partial counts race across the 16 per-DMA sem incs)
                        vector.wait_ge(s_od, 32 * c)
                    vector.tensor_copy(os_s[:, j * NCHUNK:(j + 1) * NCHUNK],
                                       banks[j][:]).then_inc(s_cp, 1)

    return nc


def _get_program():
    global _PROGRAM
    if _PROGRAM is None:
        _PROGRAM = _build_program()
    return _PROGRAM


def _host_prep(x, a_cache, b_cache, base_weight, scaling,
               q_start_loc, q_seqlens, adapter_ids, rank_offset, ranks):
    """Build the 8 per-core input maps (sharding + tiny routing gathers)."""
    x = np.asarray(x, np.float32)
    a_cache = np.asarray(a_cache, np.float32)
    b_cache = np.asarray(b_cache, np.float32)
    base_weight = np.asarray(base_weight, np.float32)
    scaling = np.asarray(scaling, np.float32)
    q_start_loc = np.asarray(q_start_loc, np.int64)
    adapter_ids = np.asarray(adapter_ids, np.int64)
    rank_offset = np.asarray(rank_offset, np.int64)
    ranks = np.asarray(ranks, np.int64)

    T = x.shape[0]
    assert T == NCORES * T_CORE
    # exact reference routing: per-token adapter, then check 512-block uniformity
    tok = np.arange(T)
    seq_idx = np.searchsorted(q_start_loc, tok, side="right") - 1
    tok_adapter = adapter_ids[seq_idx]
    blocks = tok_adapter.reshape(T // SEQ_LEN, SEQ_LEN)
    assert (blocks == blocks[:, :1]).all(), "non-uniform 512-token blocks"
    block_adapter = blocks[:, 0]  # [16]

    xb = x.astype(NP_BF16)
    # wt layout: wt[p, (c*KT + k)*512 + n] = W[c*512 + n, k*128 + p]
    wb = np.ascontiguousarray(base_weight.T).astype(NP_BF16)  # [K, N]
    wt = np.ascontiguousarray(
        wb.reshape(KT, P, NC_N, NCHUNK).transpose(1, 2, 0, 3)
    ).reshape(P, NC_N * WSLOT)

    in_maps = []
    for c in range(NCORES):
        rows = slice(c * T_CORE, (c + 1) * T_CORE)
        # xt layout: xt[p, k*1024 + t] = x[row0 + t, k*128 + p]
        xt = np.ascontiguousarray(
            xb[rows].T.reshape(KT, P, T_CORE).transpose(1, 0, 2)
        ).reshape(P, KT * T_CORE)
        bs = np.zeros((P, N), np.float32)
        xaT = np.zeros((P, T_CORE), np.float32)
        for s in range(2):  # two sequences per core
            a = int(block_adapter[2 * c + s])
            r = int(ranks[a])
            idxs = rank_offset[a, :r]
            bs[s * MAX_RANK: s * MAX_RANK + r, :] = b_cache[idxs] * scaling[a]
            # tiny rank-reduction (xa = x @ A.T) on host: [512, K] @ [K, r]
            xa = x[c * T_CORE + s * SEQ_LEN:c * T_CORE + (s + 1) * SEQ_LEN] \
                @ a_cache[idxs].T
            xaT[s * MAX_RANK: s * MAX_RANK + r,
                s * SEQ_LEN:(s + 1) * SEQ_LEN] = xa.T
        in_maps.append({"xt": xt, "wt": wt, "xad": xaT.astype(NP_BF16),
                        "bs": bs.astype(NP_BF16)})
    return in_maps


LAST_RESULT = None  # BassKernelResults of the most recent run (for profiling)


def _can_trace():
    """NTFF profiling under axon needs antenv.axon_hooks (may be shimmed by
    the caller); without it run_bass_kernel_spmd(trace=True) raises."""
    try:
        from antenv.axon_hooks import get_axon_ntff_profile_hook  # noqa: F401
        return True
    except ImportError:
        return False


def kernel(**inputs) -> np.ndarray:
    global LAST_RESULT
    import os
    nc = _get_program()
    in_maps = _host_prep(**inputs)
    trace = os.environ.get("KERNEL_TRACE") == "1" and _can_trace()
    kw = {}
    if trace:
        kw = dict(trace=True, trace_cores=list(range(NCORES)))
    res = run_bass_kernel_spmd(nc, in_maps, core_ids=list(range(NCORES)), **kw)
    LAST_RESULT = res
    out = np.empty((NCORES * T_CORE, N), np.float32)
    for c in range(NCORES):
        # out buf: [p, (cc*TT + j)*512 + n] -> out[j*128 + p, cc*512 + n]
        buf = res.results[c]["out"].reshape(P, NC_N, TT, NCHUNK)
        out[c * T_CORE:(c + 1) * T_CORE] = (
            buf.transpose(2, 0, 1, 3).reshape(T_CORE, N))
    return out


# revision 28
# speedup vs baseline: 1.0002x; 1.0002x over previous
"""Trainium2 Bass kernel for fused LoRA linear with per-sequence adapter routing.

Problem (hardcoded shapes):
  x [8192, 4096] fp32, base_weight [4096, 4096], a_cache/b_cache [512, 4096],
  16 sequences x 512 tokens, 8 adapters (rank <= 64), out [8192, 4096]:
      out = x @ base_weight.T + scaling[a(t)] * (x @ A[a(t)].T masked) @ B[a(t)]

Sharding: data-parallel over tokens. Core c handles sequences {2c, 2c+1}
(tokens [1024c, 1024c+1024)) and computes the full 4096 output features for
its tokens. Host-side prep gathers/masks/scales the per-sequence LoRA weights,
converts x/W/B to bf16, computes the tiny rank-reduction xa = x @ A.T
(<2% of total FLOPs; the 274-GFLOP base GEMM + rank-expansion stay on
device), and lays every DRAM tensor out in its exact SBUF layout so each DMA
is one large fully-contiguous transfer.

Device schedule (per core):
  - SP queue: xa (1 DMA), bs (1 DMA), xT (8 DMAs of 4 k-tiles).
  - Activation queue: w chunk stream (2-slot SBUF ring; chunk 0 split in 4
    pieces gated on xT arrival so startup loads aren't starved on the shared
    SDMA engines) interleaved with the per-chunk output stores.
  - PE: every output chunk accumulates lora first (xaT.T @ bs) then the
    32 k-tiles of the base GEMM. Chunk 0 runs k-outer across all 8 PSUM
    banks, consuming xT groups as they arrive; chunks 1..7 run j-outer/
    k-inner so each PSUM bank's stop lands ~7us before the next chunk
    reopens it -> the DVE drain copies never stall the PE.
  - DVE: PSUM -> SBUF output copies (fp32).

All matmuls bf16 (1 cycle/row, fast weight load), fp32 PSUM accumulation.
Semaphore discipline: every individually-awaited DMA has its own semaphore
(partial counts on a shared semaphore race across the 16 per-engine
increments); shared-semaphore waits only use full counts.
"""
import numpy as np

import concourse.bass as bass
import concourse.mybir as mybir
from concourse.bass_utils import run_bass_kernel_spmd

P = 128
NCORES = 8
T_CORE = 1024            # tokens per core (2 sequences)
K = 4096                 # in features
N = 4096                 # out features
KT = K // P              # 32 k-tiles
NCHUNK = 512             # psum free dim per matmul
NC_N = N // NCHUNK       # 8 n-chunks
TT = T_CORE // P         # 8 t-tiles per core
SEQ_LEN = 512
MAX_RANK = 64
KG = 4                   # k-tiles per xT DMA group
NG = KT // KG            # 8 xT groups
WSLOT = KT * NCHUNK      # w ring slot width (one n-chunk, all k)
WPIECE = WSLOT // 4      # chunk-0 w DMA piece (8 k-tiles)

F32 = mybir.dt.float32
BF16 = mybir.dt.bfloat16
NP_BF16 = mybir.dt.np(BF16)

_PROGRAM = None  # cached (nc,) build


def _build_program():
    nc = bass.Bass()
    xt_d = nc.dram_tensor("xt", [P, KT * T_CORE], BF16, kind="ExternalInput")
    wt_d = nc.dram_tensor("wt", [P, NC_N * WSLOT], BF16, kind="ExternalInput")
    xad_d = nc.dram_tensor("xad", [P, T_CORE], BF16, kind="ExternalInput")
    bs_d = nc.dram_tensor("bs", [P, N], BF16, kind="ExternalInput")
    out_d = nc.dram_tensor("out", [P, NC_N * TT * NCHUNK], F32, kind="ExternalOutput")

    from contextlib import ExitStack
    with ExitStack() as ctx:
        e = ctx.enter_context
        xT_s = e(nc.sbuf_tensor("xT_s", [P, KT * T_CORE], BF16))   # 64 KB/part
        w_s = e(nc.sbuf_tensor("w_s", [P, 2 * WSLOT], BF16))       # 64 KB/part
        bs_s = e(nc.sbuf_tensor("bs_s", [P, N], BF16))             # 8 KB/part
        xaT_s = e(nc.sbuf_tensor("xaT_s", [P, T_CORE], BF16))      # 2 KB/part
        os_s = e(nc.sbuf_tensor("os_s", [P, TT * NCHUNK], F32))    # 16 KB/part
        banks = [e(nc.psum_tensor(f"pbank{i}", [P, NCHUNK], F32)) for i in range(8)]
        s_xad = e(nc.semaphore("s_xad"))
        xt_sems = [e(nc.semaphore(f"s_xt{i}")) for i in range(NG)]
        s_bs = e(nc.semaphore("s_bs"))
        w_sems = [e(nc.semaphore(f"s_w{i}")) for i in range(2)]
        wp_sems = [e(nc.semaphore(f"s_wp{i}")) for i in range(4)]
        s_bank = e(nc.semaphore("s_bank"))
        s_cp = e(nc.semaphore("s_cp"))
        s_od = e(nc.semaphore("s_od"))
        block = e(nc.Block(no_gpsimd_drain=True))  # gpsimd entirely unused

        def xtile(k, j):
            return xT_s[:, k * T_CORE + j * P:k * T_CORE + (j + 1) * P]

        def wsl(c, k):
            base = (c % 2) * WSLOT + k * NCHUNK
            return w_s[:, base:base + NCHUNK]

        def wslot_ready(c):
            # w_sems value guaranteeing chunk c resident: slot 0 gets chunks
            # 2,4,6 (chunk 0 arrives via wp_sems pieces); slot 1 gets 1,3,5,7.
            if c % 2 == 0:
                return 16 * (c // 2)
            return 16 * ((c + 1) // 2)

        @block.sync
        def _(sync):
            sync.dma_start(out=xaT_s[:], in_=xad_d[:]).then_inc(s_xad, 16)
            sync.dma_start(out=bs_s[:], in_=bs_d[:]).then_inc(s_bs, 16)
            gw = KG * T_CORE
            for g in range(NG):
                sync.dma_start(
                    out=xT_s[:, g * gw:(g + 1) * gw],
                    in_=xt_d[:, g * gw:(g + 1) * gw],
                ).then_inc(xt_sems[g], 16)

        @block.scalar
        def _(scalar):
            # w chunk stream (2-slot ring) interleaved with output stores.
            # chunk 0 in 4 pieces, gated so the startup-critical loads aren't
            # starved on the shared SDMA engines.
            for i in range(4):
                if i > 0:
                    scalar.wait_ge(xt_sems[2 * i - 1], 16)
                scalar.dma_start(
                    out=w_s[:, i * WPIECE:(i + 1) * WPIECE],
                    in_=wt_d[:, i * WPIECE:(i + 1) * WPIECE],
                ).then_inc(wp_sems[i], 16)
            scalar.wait_ge(xt_sems[NG - 1], 16)
            ow = TT * NCHUNK
            hw_ = ow // 2

            def store(cc, piece):
                scalar.wait_ge(s_cp, cc * TT + 4 * (piece + 1))
                scalar.dma_start(
                    out=out_d[:, cc * ow + piece * hw_:cc * ow + (piece + 1) * hw_],
                    in_=os_s[:, piece * hw_:(piece + 1) * hw_],
                ).then_inc(s_od, 16)

            for c in range(1, NC_N):
                if c >= 2:
                    # slot's previous occupant (chunk c-2) fully drained
                    scalar.wait_ge(s_cp, (c - 1) * TT)
                scalar.dma_start(
                    out=w_s[:, (c % 2) * WSLOT:(c % 2 + 1) * WSLOT],
                    in_=wt_d[:, c * WSLOT:(c + 1) * WSLOT],
                ).then_inc(w_sems[c % 2], 16)
                if c >= 2:
                    store(c - 2, 0)
                    store(c - 2, 1)
            store(NC_N - 2, 0)
            store(NC_N - 2, 1)
            # last chunk: 8 finer pieces to shorten the tail; the final j=7
            # region drains in two halves (its copy is split the same way)
            qw = ow // 8
            cc = NC_N - 1
            for pq in range(7):
                scalar.wait_ge(s_cp, cc * TT + pq + 1)
                scalar.dma_start(
                    out=out_d[:, cc * ow + pq * qw:cc * ow + (pq + 1) * qw],
                    in_=os_s[:, pq * qw:(pq + 1) * qw],
                ).then_inc(s_od, 16)
            hq = qw // 2
            for h in range(2):
                scalar.wait_ge(s_cp, cc * TT + 8 + h)
                lo = 7 * qw + h * hq
                scalar.dma_start(
                    out=out_d[:, cc * ow + lo:cc * ow + lo + hq],
                    in_=os_s[:, lo:lo + hq],
                ).then_inc(s_od, 16)

        @block.tensor
        def _(tensor):
            # ---- chunk 0: lora first, then k-outer across all 8 banks,
            # consuming xT groups as they arrive ----
            tensor.wait_ge(s_xad, 16)
            tensor.wait_ge(s_bs, 16)
            for j in range(TT):
                tensor.matmul(
                    banks[j][:], lhsT=xaT_s[:, j * P:(j + 1) * P],
                    rhs=bs_s[:, 0:NCHUNK], start=True, stop=False)
            for g in range(NG):
                tensor.wait_ge(xt_sems[g], 16)
                if g % 2 == 0:
                    tensor.wait_ge(wp_sems[g // 2], 16)
                for k in range(g * KG, (g + 1) * KG):
                    for j in range(TT):
                        mm = tensor.matmul(
                            banks[j][:], lhsT=xtile(k, j), rhs=wsl(0, k),
                            start=False, stop=(k == KT - 1))
                        if k == KT - 1:
                            mm.then_inc(s_bank, 1)

            # ---- chunks 1..7 steady state: j-outer / k-inner ----
            for c in range(1, NC_N):
                tensor.wait_ge(w_sems[c % 2], wslot_ready(c))
                for j in range(TT):
                    # one wait opens two banks (DVE copies are sequential,
                    # so copy j+1 done implies copy j done; ~7us of slack)
                    if j % 2 == 0:
                        tensor.wait_ge(s_cp, (c - 1) * TT + j + 2)
                    tensor.matmul(
                        banks[j][:], lhsT=xaT_s[:, j * P:(j + 1) * P],
                        rhs=bs_s[:, c * NCHUNK:(c + 1) * NCHUNK],
                        start=True, stop=False)
                    for k in range(KT):
                        mm = tensor.matmul(
                            banks[j][:], lhsT=xtile(k, j), rhs=wsl(c, k),
                            start=False, stop=(k == KT - 1))
                    mm.then_inc(s_bank, 1)

        @block.vector
        def _(vector):
            # out copies psum -> staging
            for c in range(NC_N):
                for j in range(TT):
                    vector.wait_ge(s_bank, c * TT + j + 1)
                    if c >= 1 and j == 0:
                        # all stores through chunk c-1 done (full-count wait:
                        # partial counts race across the 16 per-DMA sem incs)
                        vector.wait_ge(s_od, 32 * c)
                    if c == NC_N - 1 and j == TT - 1:
                        # split the very last drain copy so the final store
                        # can start half a copy earlier
                        hn = NCHUNK // 2
                        vector.tensor_copy(
                            os_s[:, j * NCHUNK:j * NCHUNK + hn],
                            banks[j][:, 0:hn]).then_inc(s_cp, 1)
                        vector.tensor_copy(
                            os_s[:, j * NCHUNK + hn:(j + 1) * NCHUNK],
                            banks[j][:, hn:NCHUNK]).then_inc(s_cp, 1)
                    else:
                        vector.tensor_copy(
                            os_s[:, j * NCHUNK:(j + 1) * NCHUNK],
                            banks[j][:]).then_inc(s_cp, 1)

    return nc


def _get_program():
    global _PROGRAM
    if _PROGRAM is None:
        _PROGRAM = _build_program()
    return _PROGRAM


def _host_prep(x, a_cache, b_cache, base_weight, scaling,
               q_start_loc, q_seqlens, adapter_ids, rank_offset, ranks):
    """Build the 8 per-core input maps (sharding + tiny routing gathers)."""
    x = np.asarray(x, np.float32)
    a_cache = np.asarray(a_cache, np.float32)
    b_cache = np.asarray(b_cache, np.float32)
    base_weight = np.asarray(base_weight, np.float32)
    scaling = np.asarray(scaling, np.float32)
    q_start_loc = np.asarray(q_start_loc, np.int64)
    adapter_ids = np.asarray(adapter_ids, np.int64)
    rank_offset = np.asarray(rank_offset, np.int64)
    ranks = np.asarray(ranks, np.int64)

    T = x.shape[0]
    assert T == NCORES * T_CORE
    # exact reference routing: per-token adapter, then check 512-block uniformity
    tok = np.arange(T)
    seq_idx = np.searchsorted(q_start_loc, tok, side="right") - 1
    tok_adapter = adapter_ids[seq_idx]
    blocks = tok_adapter.reshape(T // SEQ_LEN, SEQ_LEN)
    assert (blocks == blocks[:, :1]).all(), "non-uniform 512-token blocks"
    block_adapter = blocks[:, 0]  # [16]

    xb = x.astype(NP_BF16)
    # wt layout: wt[p, (c*KT + k)*512 + n] = W[c*512 + n, k*128 + p]
    wb = np.ascontiguousarray(base_weight.T).astype(NP_BF16)  # [K, N]
    wt = np.ascontiguousarray(
        wb.reshape(KT, P, NC_N, NCHUNK).transpose(1, 2, 0, 3)
    ).reshape(P, NC_N * WSLOT)

    in_maps = []
    for c in range(NCORES):
        rows = slice(c * T_CORE, (c + 1) * T_CORE)
        # xt layout: xt[p, k*1024 + t] = x[row0 + t, k*128 + p]
        xt = np.ascontiguousarray(
            xb[rows].T.reshape(KT, P, T_CORE).transpose(1, 0, 2)
        ).reshape(P, KT * T_CORE)
        bs = np.zeros((P, N), np.float32)
        xaT = np.zeros((P, T_CORE), np.float32)
        for s in range(2):  # two sequences per core
            a = int(block_adapter[2 * c + s])
            r = int(ranks[a])
            idxs = rank_offset[a, :r]
            bs[s * MAX_RANK: s * MAX_RANK + r, :] = b_cache[idxs] * scaling[a]
            # tiny rank-reduction (xa = x @ A.T) on host: [512, K] @ [K, r]
            xa = x[c * T_CORE + s * SEQ_LEN:c * T_CORE + (s + 1) * SEQ_LEN] \
                @ a_cache[idxs].T
            xaT[s * MAX_RANK: s * MAX_RANK + r,
                s * SEQ_LEN:(s + 1) * SEQ_LEN] = xa.T
        in_maps.append({"xt": xt, "wt": wt, "xad": xaT.astype(NP_BF16),
                        "bs": bs.astype(NP_BF16)})
    return in_maps


LAST_RESULT = None  # BassKernelResults of the most recent run (for profiling)


def _can_trace():
    """NTFF profiling under axon needs antenv.axon_hooks (may be shimmed by
    the caller); without it run_bass_kernel_spmd(trace=True) raises."""
    try:
        from antenv.axon_hooks import get_axon_ntff_profile_hook  # noqa: F401
        return True
    except ImportError:
        return False


def kernel(**inputs) -> np.ndarray:
    global LAST_RESULT
    import os
    nc = _get_program()
    in_maps = _host_prep(**inputs)
    trace = os.environ.get("KERNEL_TRACE") == "1" and _can_trace()
    kw = {}
    if trace:
        kw = dict(trace=True, trace_cores=list(range(NCORES)))
    res = run_bass_kernel_spmd(nc, in_maps, core_ids=list(range(NCORES)), **kw)
    LAST_RESULT = res
    out = np.empty((NCORES * T_CORE, N), np.float32)
    for c in range(NCORES):
        # out buf: [p, (cc*TT + j)*512 + n] -> out[j*128 + p, cc*512 + n]
        buf = res.results[c]["out"].reshape(P, NC_N, TT, NCHUNK)
        out[c * T_CORE:(c + 1) * T_CORE] = (
            buf.transpose(2, 0, 1, 3).reshape(T_CORE, N))
    return out


# revision 29
# speedup vs baseline: 1.0056x; 1.0054x over previous
"""Trainium2 Bass kernel for fused LoRA linear with per-sequence adapter routing.

Problem (hardcoded shapes):
  x [8192, 4096] fp32, base_weight [4096, 4096], a_cache/b_cache [512, 4096],
  16 sequences x 512 tokens, 8 adapters (rank <= 64), out [8192, 4096]:
      out = x @ base_weight.T + scaling[a(t)] * (x @ A[a(t)].T masked) @ B[a(t)]

Sharding: data-parallel over tokens. Core c handles sequences {2c, 2c+1}
(tokens [1024c, 1024c+1024)) and computes the full 4096 output features for
its tokens. Host-side prep gathers/masks/scales the per-sequence LoRA weights,
converts x/W/B to bf16, computes the tiny rank-reduction xa = x @ A.T
(<2% of total FLOPs; the 274-GFLOP base GEMM + rank-expansion stay on
device), and lays every DRAM tensor out in its exact SBUF layout so each DMA
is one large fully-contiguous transfer.

Device schedule (per core):
  - SP queue: xa (1 DMA), bs (1 DMA), xT (8 DMAs of 4 k-tiles).
  - Activation queue: w chunk stream (2-slot SBUF ring; chunk 0 split in 4
    pieces gated on xT arrival so startup loads aren't starved on the shared
    SDMA engines) interleaved with the per-chunk output stores.
  - PE: every output chunk accumulates lora first (xaT.T @ bs) then the
    32 k-tiles of the base GEMM. Chunk 0 runs k-outer across all 8 PSUM
    banks, consuming xT groups as they arrive; chunks 1..7 run j-outer/
    k-inner so each PSUM bank's stop lands ~7us before the next chunk
    reopens it -> the DVE drain copies never stall the PE.
  - DVE: PSUM -> SBUF output copies (fp32).

All matmuls bf16 (1 cycle/row, fast weight load), fp32 PSUM accumulation.
Semaphore discipline: every individually-awaited DMA has its own semaphore
(partial counts on a shared semaphore race across the 16 per-engine
increments); shared-semaphore waits only use full counts.
"""
import numpy as np

import concourse.bass as bass
import concourse.mybir as mybir
from concourse.bass_utils import run_bass_kernel_spmd

P = 128
NCORES = 8
T_CORE = 1024            # tokens per core (2 sequences)
K = 4096                 # in features
N = 4096                 # out features
KT = K // P              # 32 k-tiles
NCHUNK = 512             # psum free dim per matmul
NC_N = N // NCHUNK       # 8 n-chunks
TT = T_CORE // P         # 8 t-tiles per core
SEQ_LEN = 512
MAX_RANK = 64
KG = 4                   # k-tiles per xT DMA group
NG = KT // KG            # 8 xT groups
WSLOT = KT * NCHUNK      # w ring slot width (one n-chunk, all k)
WPIECE = WSLOT // 4      # chunk-0 w DMA piece (8 k-tiles)

F32 = mybir.dt.float32
BF16 = mybir.dt.bfloat16
NP_BF16 = mybir.dt.np(BF16)

_PROGRAM = None  # cached (nc,) build


def _build_program():
    nc = bass.Bass()
    xt_d = nc.dram_tensor("xt", [P, KT * T_CORE], BF16, kind="ExternalInput")
    wt_d = nc.dram_tensor("wt", [P, NC_N * WSLOT], BF16, kind="ExternalInput")
    xad_d = nc.dram_tensor("xad", [P, T_CORE], BF16, kind="ExternalInput")
    bs_d = nc.dram_tensor("bs", [P, N], BF16, kind="ExternalInput")
    out_d = nc.dram_tensor("out", [P, NC_N * TT * NCHUNK], F32, kind="ExternalOutput")

    from contextlib import ExitStack
    with ExitStack() as ctx:
        e = ctx.enter_context
        xT_s = e(nc.sbuf_tensor("xT_s", [P, KT * T_CORE], BF16))   # 64 KB/part
        w_s = e(nc.sbuf_tensor("w_s", [P, 2 * WSLOT], BF16))       # 64 KB/part
        bs_s = e(nc.sbuf_tensor("bs_s", [P, N], BF16))             # 8 KB/part
        xaT_s = e(nc.sbuf_tensor("xaT_s", [P, T_CORE], BF16))      # 2 KB/part
        os_s = e(nc.sbuf_tensor("os_s", [P, TT * NCHUNK], F32))    # 16 KB/part
        banks = [e(nc.psum_tensor(f"pbank{i}", [P, NCHUNK], F32)) for i in range(8)]
        s_xad = e(nc.semaphore("s_xad"))
        xt_sems = [e(nc.semaphore(f"s_xt{i}")) for i in range(NG)]
        s_bs = e(nc.semaphore("s_bs"))
        w_sems = [e(nc.semaphore(f"s_w{i}")) for i in range(2)]
        wp_sems = [e(nc.semaphore(f"s_wp{i}")) for i in range(4)]
        s_bank = e(nc.semaphore("s_bank"))
        s_cp = e(nc.semaphore("s_cp"))
        s_od = e(nc.semaphore("s_od"))
        block = e(nc.Block(no_gpsimd_drain=True))  # gpsimd entirely unused

        def xtile(k, j):
            return xT_s[:, k * T_CORE + j * P:k * T_CORE + (j + 1) * P]

        def wsl(c, k):
            base = (c % 2) * WSLOT + k * NCHUNK
            return w_s[:, base:base + NCHUNK]

        def wslot_ready(c):
            # w_sems value guaranteeing chunk c resident: slot 0 gets chunks
            # 2,4,6 (chunk 0 arrives via wp_sems pieces); slot 1 gets 1,3,5,7.
            if c % 2 == 0:
                return 16 * (c // 2)
            return 16 * ((c + 1) // 2)

        @block.sync
        def _(sync):
            sync.dma_start(out=xaT_s[:], in_=xad_d[:]).then_inc(s_xad, 16)
            sync.dma_start(out=bs_s[:], in_=bs_d[:]).then_inc(s_bs, 16)
            gw = KG * T_CORE
            for g in range(NG):
                sync.dma_start(
                    out=xT_s[:, g * gw:(g + 1) * gw],
                    in_=xt_d[:, g * gw:(g + 1) * gw],
                ).then_inc(xt_sems[g], 16)

        @block.scalar
        def _(scalar):
            # w chunk stream (2-slot ring) interleaved with output stores.
            # chunk 0 in 4 pieces, gated so the startup-critical loads aren't
            # starved on the shared SDMA engines.
            for i in range(4):
                if i > 0:
                    scalar.wait_ge(xt_sems[2 * i - 1], 16)
                scalar.dma_start(
                    out=w_s[:, i * WPIECE:(i + 1) * WPIECE],
                    in_=wt_d[:, i * WPIECE:(i + 1) * WPIECE],
                ).then_inc(wp_sems[i], 16)
            scalar.wait_ge(xt_sems[NG - 1], 16)
            ow = TT * NCHUNK
            hw_ = ow // 2

            def store(cc, piece):
                scalar.wait_ge(s_cp, cc * TT + 4 * (piece + 1))
                scalar.dma_start(
                    out=out_d[:, cc * ow + piece * hw_:cc * ow + (piece + 1) * hw_],
                    in_=os_s[:, piece * hw_:(piece + 1) * hw_],
                ).then_inc(s_od, 16)

            for c in range(1, NC_N):
                if c >= 2:
                    # slot's previous occupant (chunk c-2) fully drained
                    scalar.wait_ge(s_cp, (c - 1) * TT)
                scalar.dma_start(
                    out=w_s[:, (c % 2) * WSLOT:(c % 2 + 1) * WSLOT],
                    in_=wt_d[:, c * WSLOT:(c + 1) * WSLOT],
                ).then_inc(w_sems[c % 2], 16)
                if c >= 2:
                    store(c - 2, 0)
                    store(c - 2, 1)
            store(NC_N - 2, 0)
            store(NC_N - 2, 1)
            # last chunk: 8 finer pieces to shorten the tail
            qw = ow // 8
            cc = NC_N - 1
            for pq in range(8):
                scalar.wait_ge(s_cp, cc * TT + pq + 1)
                scalar.dma_start(
                    out=out_d[:, cc * ow + pq * qw:cc * ow + (pq + 1) * qw],
                    in_=os_s[:, pq * qw:(pq + 1) * qw],
                ).then_inc(s_od, 16)

        @block.tensor
        def _(tensor):
            # ---- chunk 0: lora first, then k-outer across all 8 banks,
            # consuming xT groups as they arrive ----
            tensor.wait_ge(s_xad, 16)
            tensor.wait_ge(s_bs, 16)
            for j in range(TT):
                tensor.matmul(
                    banks[j][:], lhsT=xaT_s[:, j * P:(j + 1) * P],
                    rhs=bs_s[:, 0:NCHUNK], start=True, stop=False)
            for g in range(NG):
                tensor.wait_ge(xt_sems[g], 16)
                if g % 2 == 0:
                    tensor.wait_ge(wp_sems[g // 2], 16)
                for k in range(g * KG, (g + 1) * KG):
                    for j in range(TT):
                        mm = tensor.matmul(
                            banks[j][:], lhsT=xtile(k, j), rhs=wsl(0, k),
                            start=False, stop=(k == KT - 1))
                        if k == KT - 1:
                            mm.then_inc(s_bank, 1)

            # ---- chunks 1..7 steady state: j-outer / k-inner ----
            for c in range(1, NC_N):
                tensor.wait_ge(w_sems[c % 2], wslot_ready(c))
                for j in range(TT):
                    tensor.wait_ge(s_cp, (c - 1) * TT + j + 1)
                    tensor.matmul(
                        banks[j][:], lhsT=xaT_s[:, j * P:(j + 1) * P],
                        rhs=bs_s[:, c * NCHUNK:(c + 1) * NCHUNK],
                        start=True, stop=False)
                    for k in range(KT):
                        mm = tensor.matmul(
                            banks[j][:], lhsT=xtile(k, j), rhs=wsl(c, k),
                            start=False, stop=(k == KT - 1))
                    mm.then_inc(s_bank, 1)

        @block.vector
        def _(vector):
            # out copies psum -> staging
            for c in range(NC_N):
                for j in range(TT):
                    vector.wait_ge(s_bank, c * TT + j + 1)
                    if c >= 1 and j == 0:
                        # all stores through chunk c-1 done (full-count wait:
                        # partial counts race across the 16 per-DMA sem incs)
                        vector.wait_ge(s_od, 32 * c)
                    vector.tensor_copy(os_s[:, j * NCHUNK:(j + 1) * NCHUNK],
                                       banks[j][:]).then_inc(s_cp, 1)

    return nc


def _get_program():
    global _PROGRAM
    if _PROGRAM is None:
        _PROGRAM = _build_program()
    return _PROGRAM


def _host_prep(x, a_cache, b_cache, base_weight, scaling,
               q_start_loc, q_seqlens, adapter_ids, rank_offset, ranks):
    """Build the 8 per-core input maps (sharding + tiny routing gathers)."""
    x = np.asarray(x, np.float32)
    a_cache = np.asarray(a_cache, np.float32)
    b_cache = np.asarray(b_cache, np.float32)
    base_weight = np.asarray(base_weight, np.float32)
    scaling = np.asarray(scaling, np.float32)
    q_start_loc = np.asarray(q_start_loc, np.int64)
    adapter_ids = np.asarray(adapter_ids, np.int64)
    rank_offset = np.asarray(rank_offset, np.int64)
    ranks = np.asarray(ranks, np.int64)

    T = x.shape[0]
    assert T == NCORES * T_CORE
    # exact reference routing: per-token adapter, then check 512-block uniformity
    tok = np.arange(T)
    seq_idx = np.searchsorted(q_start_loc, tok, side="right") - 1
    tok_adapter = adapter_ids[seq_idx]
    blocks = tok_adapter.reshape(T // SEQ_LEN, SEQ_LEN)
    assert (blocks == blocks[:, :1]).all(), "non-uniform 512-token blocks"
    block_adapter = blocks[:, 0]  # [16]

    xb = x.astype(NP_BF16)
    # wt layout: wt[p, (c*KT + k)*512 + n] = W[c*512 + n, k*128 + p]
    wb = np.ascontiguousarray(base_weight.T).astype(NP_BF16)  # [K, N]
    wt = np.ascontiguousarray(
        wb.reshape(KT, P, NC_N, NCHUNK).transpose(1, 2, 0, 3)
    ).reshape(P, NC_N * WSLOT)

    in_maps = []
    for c in range(NCORES):
        rows = slice(c * T_CORE, (c + 1) * T_CORE)
        # xt layout: xt[p, k*1024 + t] = x[row0 + t, k*128 + p]
        xt = np.ascontiguousarray(
            xb[rows].T.reshape(KT, P, T_CORE).transpose(1, 0, 2)
        ).reshape(P, KT * T_CORE)
        bs = np.zeros((P, N), np.float32)
        xaT = np.zeros((P, T_CORE), np.float32)
        for s in range(2):  # two sequences per core
            a = int(block_adapter[2 * c + s])
            r = int(ranks[a])
            idxs = rank_offset[a, :r]
            bs[s * MAX_RANK: s * MAX_RANK + r, :] = b_cache[idxs] * scaling[a]
            # tiny rank-reduction (xa = x @ A.T) on host: [512, K] @ [K, r]
            xa = x[c * T_CORE + s * SEQ_LEN:c * T_CORE + (s + 1) * SEQ_LEN] \
                @ a_cache[idxs].T
            xaT[s * MAX_RANK: s * MAX_RANK + r,
                s * SEQ_LEN:(s + 1) * SEQ_LEN] = xa.T
        in_maps.append({"xt": xt, "wt": wt, "xad": xaT.astype(NP_BF16),
                        "bs": bs.astype(NP_BF16)})
    return in_maps


LAST_RESULT = None  # BassKernelResults of the most recent run (for profiling)


def _can_trace():
    """NTFF profiling under axon needs antenv.axon_hooks (may be shimmed by
    the caller); without it run_bass_kernel_spmd(trace=True) raises."""
    try:
        from antenv.axon_hooks import get_axon_ntff_profile_hook  # noqa: F401
        return True
    except ImportError:
        return False


def kernel(**inputs) -> np.ndarray:
    global LAST_RESULT
    import os
    nc = _get_program()
    in_maps = _host_prep(**inputs)
    trace = os.environ.get("KERNEL_TRACE") == "1" and _can_trace()
    kw = {}
    if trace:
        kw = dict(trace=True, trace_cores=list(range(NCORES)))
    res = run_bass_kernel_spmd(nc, in_maps, core_ids=list(range(NCORES)), **kw)
    LAST_RESULT = res
    out = np.empty((NCORES * T_CORE, N), np.float32)
    for c in range(NCORES):
        # out buf: [p, (cc*TT + j)*512 + n] -> out[j*128 + p, cc*512 + n]
        buf = res.results[c]["out"].reshape(P, NC_N, TT, NCHUNK)
        out[c * T_CORE:(c + 1) * T_CORE] = (
            buf.transpose(2, 0, 1, 3).reshape(T_CORE, N))
    return out
